# revision 1
# baseline (speedup 1.0000x reference)
"""Trainium2 Bass kernel for nn_ConsistencyLoss.

loss = -mean(masked_select(cos_sim(mom[b,:,m], base[b,:,n]), mask))

Reformulation so the 268MB int32 mask is streamed through the tensor engine
exactly once (memory-bound regime):

    masked_sum = sum_{b,m,n} M[b,m,n] * (mhat_m . bhat_n)
               = sum_{b,c,n} bhat[c,n] * W[c,n],   W = mhatT.T @ M   (contract over m)
    loss       = -masked_sum / sum(M)

Sharding: 8 cores = (batch b in 0..3) x (half of the m rows). Each core:
  - mom slice [C=128, 2048] -> transpose -> normalize -> split hi/lo bf16
    (0/1 mask is exact in bf16; hi+lo matmul pair recovers fp32 accuracy)
  - mask slice [2048, 4096] int32, streamed as 16 x [128,4096] cast-DMAs
    (int32 -> bf16 in the DMA engine) feeding rhs of accumulating matmuls
  - W accumulated in 8 PSUM banks [128 c, 512 n]
  - tail: Q = base .* W, V[n] = sum_c Q (ones-matmul), masked_sum = V . rnb
  - count from per-tile DVE reduces of the bf16 mask (exact in fp32)
Host combines 8 (sum, count) pairs.
"""

import sys

for _p in ("/opt/trn_rl_repo",):
    if _p not in sys.path:
        sys.path.insert(0, _p)

import numpy as np

B, C, HW = 4, 128, 4096          # batch, channels, H*W
M_LOC = HW // 2                  # momentum rows per core
N_CORES = 8

_RUNNER = None


def build_nc(n=HW, m_loc=M_LOC, n_cores=N_CORES, reps=1):
    """Build the per-core Bass module. n = base pixels, m_loc = local momentum rows."""
    import concourse.bass as bass
    import concourse.mybir as mybir
    import concourse.tile as tile
    from concourse import bacc
    from concourse.masks import make_identity
    from concourse.bass import ts

    f32 = mybir.dt.float32
    bf16 = mybir.dt.bfloat16
    i32 = mybir.dt.int32
    T = m_loc // 128             # mask row tiles
    NB = n // 512                # 512-wide n blocks (one PSUM bank each)
    assert NB <= 8

    nc = bacc.Bacc("TRN2", target_bir_lowering=False, debug=False,
                   num_devices=n_cores)
    base_d = nc.declare_dram_parameter("base", [128, n], f32, isOutput=False)
    mom_d = nc.declare_dram_parameter("mom", [128, m_loc], f32, isOutput=False)
    mask_d = nc.declare_dram_parameter("mask", [m_loc, n], i32, isOutput=False)
    out_d = nc.declare_dram_parameter("out", [1, 2 * reps], f32, isOutput=True)
    mask3 = mask_d.rearrange("(t p) n -> t p n", p=128)

    with tile.TileContext(nc) as tc:
        from contextlib import ExitStack
        with ExitStack() as ctx:
            sing = ctx.enter_context(tc.tile_pool(name="sing", bufs=1))
            sb = ctx.enter_context(tc.tile_pool(name="sb", bufs=1))
            mkp = ctx.enter_context(tc.tile_pool(name="mk", bufs=3))
            smallp = ctx.enter_context(tc.tile_pool(name="small", bufs=2))
            qp = ctx.enter_context(tc.tile_pool(name="qp", bufs=1))

            ident = sing.tile([128, 128], f32)
            make_identity(nc, ident)
            onescol = sing.tile([128, 1], f32)
            nc.vector.memset(onescol, 1.0)

            outpair = sb.tile([1, 2 * reps], f32)

            for rep in range(reps):
                sb_base = sb.tile([128, n], f32, tag="sb_base")
                nc.sync.dma_start(out=sb_base[:], in_=base_d[:])
                sb_mom = sb.tile([128, m_loc], f32, tag="sb_mom")
                nc.sync.dma_start(out=sb_mom[:], in_=mom_d[:])

                mh_hi = sb.tile([128, T, 128], bf16, tag="mh_hi")
                mh_lo = sb.tile([128, T, 128], bf16, tag="mh_lo")
                nsq = sb.tile([128, T], f32, tag="nsq")
                rn_a = sb.tile([128, T], f32, tag="rn_a")
                rn = sb.tile([128, T], f32, tag="rn")
                cntc = sb.tile([128, T], f32, tag="cntc")

                # ---- phase 0a: transpose mom tiles, norms, hi/lo split ----
                with tc.tile_pool(name=f"ps0_{rep}", bufs=3, space="PSUM") as ps0:
                    for t in range(T):
                        tp = ps0.tile([128, 128], f32, tag="tp")
                        nc.tensor.transpose(tp[:], sb_mom[:, ts(t, 128)], ident[:])
                        tpc = smallp.tile([128, 128], f32, tag="tpc")
                        nc.scalar.copy(out=tpc[:], in_=tp[:])
                        sq = smallp.tile([128, 128], f32, tag="sq")
                        nc.vector.tensor_mul(sq[:], tpc[:], tpc[:])
                        nc.vector.reduce_sum(out=nsq[:, t:t + 1], in_=sq[:],
                                             axis=mybir.AxisListType.X)
                        nc.vector.reciprocal(out=rn_a[:, t:t + 1], in_=nsq[:, t:t + 1])
                        nc.scalar.activation(out=rn[:, t:t + 1], in_=rn_a[:, t:t + 1],
                                             func=mybir.ActivationFunctionType.Sqrt)
                        mh32 = smallp.tile([128, 128], f32, tag="mh32")
                        nc.vector.tensor_scalar_mul(mh32[:], tpc[:], rn[:, t:t + 1])
                        nc.vector.tensor_copy(out=mh_hi[:, t, :], in_=mh32[:])
                        nc.vector.tensor_sub(mh_lo[:, t, :], mh32[:], mh_hi[:, t, :])

                # ---- phase 0b: norm_b (column norms of base) ----
                sqb = qp.tile([128, n], f32, tag="sqb")
                nc.vector.tensor_mul(sqb[:], sb_base[:], sb_base[:])
                nb2_row = sb.tile([1, n], f32, tag="nb2")
                rnb_a = sb.tile([1, n], f32, tag="rnb_a")
                rnb_row = sb.tile([1, n], f32, tag="rnb")
                with tc.tile_pool(name=f"psnb_{rep}", bufs=2, space="PSUM") as psnb:
                    for nb in range(NB):
                        pnb = psnb.tile([1, 512], f32, tag="pnb")
                        nc.tensor.matmul(pnb[:], onescol[:], sqb[:, ts(nb, 512)],
                                         start=True, stop=True)
                        nc.vector.tensor_copy(out=nb2_row[0:1, ts(nb, 512)], in_=pnb[:])
                nc.vector.reciprocal(out=rnb_a[:], in_=nb2_row[:])
                nc.scalar.activation(out=rnb_row[:], in_=rnb_a[:],
                                     func=mybir.ActivationFunctionType.Sqrt)

                # ---- phase 1: stream mask, accumulate W in PSUM ----
                with tc.tile_pool(name=f"psW_{rep}", bufs=1, space="PSUM") as psW:
                    Wb = [psW.tile([128, 512], f32, tag=f"w{nb}", name=f"w{nb}")
                          for nb in range(NB)]
                    for t in range(T):
                        mk = mkp.tile([128, n], bf16, tag="mk")
                        nc.gpsimd.dma_start(out=mk[:], in_=mask3[t])
                        nc.vector.reduce_sum(out=cntc[:, t:t + 1], in_=mk[:],
                                             axis=mybir.AxisListType.X)
                        for part, mh in ((0, mh_hi), (1, mh_lo)):
                            for nb in range(NB):
                                nc.tensor.matmul(
                                    Wb[nb][:], mh[:, t, :], mk[:, ts(nb, 512)],
                                    start=(t == 0 and part == 0),
                                    stop=(t == T - 1 and part == 1))

                    # ---- phase 2a: Q = base .* W (frees PSUM banks) ----
                    qs = []
                    for nb in range(NB):
                        q = qp.tile([128, 512], f32, tag=f"q{nb}")
                        nc.vector.tensor_mul(q[:], Wb[nb][:], sb_base[:, ts(nb, 512)])
                        qs.append(q)

                # ---- phase 2b: V[n] = sum_c Q, masked_sum, count ----
                v_row = sb.tile([1, n], f32, tag="v_row")
                with tc.tile_pool(name=f"psV_{rep}", bufs=2, space="PSUM") as psV:
                    for nb in range(NB):
                        pv = psV.tile([1, 512], f32, tag="pv")
                        nc.tensor.matmul(pv[:], onescol[:], qs[nb][:],
                                         start=True, stop=True)
                        nc.vector.tensor_copy(out=v_row[0:1, ts(nb, 512)], in_=pv[:])
                    vs = sb.tile([1, n], f32, tag="vs")
                    nc.vector.tensor_mul(vs[:], v_row[:], rnb_row[:])
                    nc.vector.reduce_sum(out=outpair[0:1, 2 * rep:2 * rep + 1],
                                         in_=vs[:], axis=mybir.AxisListType.X)
                    cnt1 = sb.tile([128, 1], f32, tag="cnt1")
                    nc.vector.reduce_sum(out=cnt1[:], in_=cntc[:],
                                         axis=mybir.AxisListType.X)
                    pc = psV.tile([1, 1], f32, tag="pc")
                    nc.tensor.matmul(pc[:], onescol[:], cnt1[:], start=True, stop=True)
                    nc.vector.tensor_copy(out=outpair[0:1, 2 * rep + 1:2 * rep + 2],
                                          in_=pc[:])

            nc.sync.dma_start(out=out_d[:], in_=outpair[:])

    nc.compile()
    return nc


class SpmdRunner:
    """Compile-once PJRT runner; keeps staged inputs on device."""

    def __init__(self, nc, n_cores):
        import jax
        from jax.sharding import Mesh, PartitionSpec
        from jax.experimental.shard_map import shard_map
        import concourse.mybir as mybir
        from concourse.bass2jax import (_bass_exec_p, install_neuronx_cc_hook,
                                        partition_id_tensor)
        install_neuronx_cc_hook()
        self.jax = jax
        self.PartitionSpec = PartitionSpec
        self.n_cores = n_cores
        in_names, out_names, out_avals, zero_outs = [], [], [], []
        partition_name = (nc.partition_id_tensor.name
                          if nc.partition_id_tensor else None)
        for alloc in nc.m.functions[0].allocations:
            if not isinstance(alloc, mybir.MemoryLocationSet):
                continue
            name = alloc.memorylocations[0].name
            if alloc.kind == "ExternalInput":
                if name != partition_name:
                    in_names.append(name)
            elif alloc.kind == "ExternalOutput":
                out_names.append(name)
                shape = tuple(alloc.tensor_shape)
                dtype = mybir.dt.np(alloc.dtype)
                out_avals.append(jax.core.ShapedArray(shape, dtype))
                zero_outs.append(np.zeros(shape, dtype))
        self.in_names, self.out_names = in_names, out_names
        self.zero_outs = zero_outs
        n_params = len(in_names)
        all_in_names = in_names + out_names
        if partition_name is not None:
            all_in_names.append(partition_name)

        def _body(*args):
            operands = list(args)
            if partition_name is not None:
                operands.append(partition_id_tensor())
            outs = _bass_exec_p.bind(
                *operands,
                out_avals=tuple(out_avals),
                in_names=tuple(all_in_names),
                out_names=tuple(out_names),
                lowering_input_output_aliases=(),
                sim_require_finite=True,
                sim_require_nnan=True,
                nc=nc,
            )
            return tuple(outs)

        devices = jax.devices()[:n_cores]
        self.mesh = Mesh(np.asarray(devices), ("core",))
        in_specs = (PartitionSpec("core"),) * (n_params + len(out_names))
        out_specs = (PartitionSpec("core"),) * len(out_names)
        self.fn = jax.jit(shard_map(_body, mesh=self.mesh, in_specs=in_specs,
                                    out_specs=out_specs, check_rep=False))

    def stage(self, in_maps):
        from jax.sharding import NamedSharding
        args = []
        for name in self.in_names:
            glob = np.concatenate([np.asarray(m[name]) for m in in_maps], axis=0)
            args.append(self.jax.device_put(
                glob, NamedSharding(self.mesh, self.PartitionSpec("core"))))
        for z in self.zero_outs:
            glob = np.concatenate([z] * self.n_cores, axis=0)
            args.append(self.jax.device_put(
                glob, NamedSharding(self.mesh, self.PartitionSpec("core"))))
        return args

    def run(self, args):
        outs = self.fn(*args)
        self.jax.block_until_ready(outs)
        return outs

    def results(self, outs):
        res = [dict() for _ in range(self.n_cores)]
        for i, name in enumerate(self.out_names):
            glob = np.asarray(outs[i])
            per = np.split(glob, self.n_cores, axis=0)
            for c in range(self.n_cores):
                res[c][name] = per[c]
        return res


def make_in_maps(en_base, en_momentum, matrix):
    """Slice full inputs per core: core k -> (batch k//2, m-half k%2)."""
    in_maps = []
    for k in range(N_CORES):
        b, h = k // 2, k % 2
        base = np.ascontiguousarray(en_base[b].reshape(C, HW))
        mom = np.ascontiguousarray(
            en_momentum[b].reshape(C, HW)[:, h * M_LOC:(h + 1) * M_LOC])
        mask = matrix[b, h * M_LOC:(h + 1) * M_LOC, :]
        in_maps.append({"base": base, "mom": mom, "mask": mask})
    return in_maps


def _get_runner():
    global _RUNNER
    if _RUNNER is None:
        nc = build_nc()
        _RUNNER = SpmdRunner(nc, N_CORES)
    return _RUNNER


def kernel(en_base, en_momentum, matrix):
    runner = _get_runner()
    args = runner.stage(make_in_maps(en_base, en_momentum, matrix))
    res = runner.results(runner.run(args))
    tot = np.zeros(2, dtype=np.float64)
    for c in range(N_CORES):
        tot += res[c]["out"][0, :2].astype(np.float64)
    loss = -(tot[0] / tot[1])
    return np.array(loss, dtype=np.float32)



# revision 5
# speedup vs baseline: 1.0440x; 1.0440x over previous
"""Trainium2 Bass kernel for nn_ConsistencyLoss.

loss = -mean(masked_select(cos_sim(mom[b,:,m], base[b,:,n]), mask))

Reformulation so the 268MB int32 mask is streamed through the tensor engine
exactly once (memory-bound regime):

    masked_sum = sum_{b,m,n} M[b,m,n] * (mhat_m . bhat_n)
               = sum_{b,c,n} bhat[c,n] * W[c,n],   W = mhatT.T @ M   (contract over m)
    loss       = -masked_sum / sum(M)

Sharding: 8 cores = (batch b in 0..3) x (half of the m rows). Each core:
  - mom slice [C=128, 2048] -> 16 PE transposes into one 4-bank PSUM tile,
    one batched DVE square+reduce for the row norms, then 16 ACT copies that
    fold the 1/||mom_m|| scale into the PSUM->SBUF bf16 cast (mhat in bf16
    costs ~1e-3 rel err on the loss, far under the 2e-2 gate)
  - mask slice [2048, 4096] int32, streamed as 16 x [128,4096] cast-DMAs
    (int32 -> bf16 in the DMA engine, 8-deep buffer pool) feeding rhs of
    accumulating matmuls; W accumulated in 8 PSUM banks [128 c, 512 n]
  - tail: Q = base .* W (bf16), V[n] = sum_c Q (ones-matmul),
    masked_sum = (V .* 1/||base_n||) summed; count from per-tile DVE reduces
    of the bf16 mask (exact in fp32)
Host combines 8 (sum, count) pairs.
"""

import sys

for _p in ("/opt/trn_rl_repo",):
    if _p not in sys.path:
        sys.path.insert(0, _p)

import numpy as np

B, C, HW = 4, 128, 4096          # batch, channels, H*W
M_LOC = HW // 2                  # momentum rows per core
N_CORES = 8

_RUNNER = None


def build_nc(n=HW, m_loc=M_LOC, n_cores=N_CORES, reps=1):
    """Build the per-core Bass module. n = base pixels, m_loc = local momentum rows."""
    import concourse.bass as bass
    import concourse.mybir as mybir
    import concourse.tile as tile
    from concourse import bacc
    from concourse.masks import make_identity
    from concourse.bass import ts

    f32 = mybir.dt.float32
    bf16 = mybir.dt.bfloat16
    i32 = mybir.dt.int32
    T = m_loc // 128             # mask row tiles
    NB = n // 512                # 512-wide n blocks (one PSUM bank each)
    assert NB <= 8

    nc = bacc.Bacc("TRN2", target_bir_lowering=False, debug=False,
                   num_devices=n_cores)
    base_d = nc.declare_dram_parameter("base", [128, n], f32, isOutput=False)
    mom_d = nc.declare_dram_parameter("mom", [128, m_loc], f32, isOutput=False)
    mask_d = nc.declare_dram_parameter("mask", [m_loc, n], i32, isOutput=False)
    out_d = nc.declare_dram_parameter("out", [1, 2 * reps], f32, isOutput=True)
    mask3 = mask_d.rearrange("(t p) n -> t p n", p=128)

    with tile.TileContext(nc) as tc:
        from contextlib import ExitStack
        with ExitStack() as ctx:
            ctx.enter_context(nc.allow_low_precision(
                reason="bf16 intermediates validated vs fp32 reference (~1e-3)"))
            sing = ctx.enter_context(tc.tile_pool(name="sing", bufs=1))
            sb = ctx.enter_context(tc.tile_pool(name="sb", bufs=2))
            rows = ctx.enter_context(tc.tile_pool(name="rows", bufs=1))
            mkp = ctx.enter_context(tc.tile_pool(name="mk", bufs=8))
            qp = ctx.enter_context(tc.tile_pool(name="qp", bufs=1))

            ident = sing.tile([128, 128], f32)
            make_identity(nc, ident)
            onescol = sing.tile([128, 1], f32)
            nc.vector.memset(onescol, 1.0)
            ones_bf = sing.tile([128, 1], bf16)
            nc.vector.memset(ones_bf, 1.0)

            outpair = sing.tile([1, 2 * reps], f32)

            for rep in range(reps):
                sb_base = sb.tile([128, n], f32, tag="sb_base")
                nc.sync.dma_start(out=sb_base[:], in_=base_d[:])
                sb_mom = sb.tile([128, m_loc], f32, tag="sb_mom")
                nc.sync.dma_start(out=sb_mom[:], in_=mom_d[:])

                mh_hi = sb.tile([128, T, 128], bf16, tag="mh_hi")
                cntc = sb.tile([128, T], f32, tag="cntc")

                # ---- phase 0a: batched mom transpose + row norms ----
                with tc.tile_pool(name=f"ps0_{rep}", bufs=1, space="PSUM") as ps0:
                    tp_all = ps0.tile([128, T, 128], f32, tag="tp_all")
                    for t in range(T):
                        nc.tensor.transpose(tp_all[:, t, :], sb_mom[:, ts(t, 128)],
                                            ident[:])
                    sqd = qp.tile([128, 128], bf16, tag="sqd")
                    nsq = sb.tile([128, T], f32, tag="nsq")
                    for t in range(T):
                        nc.scalar.activation(out=sqd[:], in_=tp_all[:, t, :],
                                             func=mybir.ActivationFunctionType.Square,
                                             accum_out=nsq[:, t:t + 1])
                    rn_a = sb.tile([128, T], f32, tag="rn_a")
                    nc.vector.reciprocal(out=rn_a[:], in_=nsq[:])
                    rn = sb.tile([128, T], f32, tag="rn")
                    nc.scalar.activation(out=rn[:], in_=rn_a[:],
                                         func=mybir.ActivationFunctionType.Sqrt)
                    for t in range(T):
                        nc.scalar.mul(mh_hi[:, t, :], tp_all[:, t, :], rn[:, t:t + 1])

                # ---- phase 0b: norm_b (column norms of base, bf16 squares) ----
                sqb = qp.tile([128, n], bf16, tag="sqb")
                nc.scalar.activation(out=sqb[:], in_=sb_base[:],
                                     func=mybir.ActivationFunctionType.Square)
                nb2_row = rows.tile([1, n], f32, tag="nb2")
                rnb_a = rows.tile([1, n], bf16, tag="rnb_a")
                rnb_row = rows.tile([1, n], bf16, tag="rnb")
                with tc.tile_pool(name=f"psnb_{rep}", bufs=2, space="PSUM") as psnb:
                    for nb in range(NB):
                        pnb = psnb.tile([1, 512], f32, tag="pnb")
                        nc.tensor.matmul(pnb[:], ones_bf[:], sqb[:, ts(nb, 512)],
                                         start=True, stop=True)
                        nc.vector.tensor_copy(out=nb2_row[0:1, ts(nb, 512)], in_=pnb[:])
                nc.vector.reciprocal(out=rnb_a[:], in_=nb2_row[:])
                nc.scalar.activation(out=rnb_row[:], in_=rnb_a[:],
                                     func=mybir.ActivationFunctionType.Sqrt)

                # ---- phase 1: stream mask, accumulate W in PSUM ----
                with tc.tile_pool(name=f"psW_{rep}", bufs=1, space="PSUM") as psW:
                    Wb = [psW.tile([128, 512], f32, tag=f"w{nb}", name=f"w{nb}")
                          for nb in range(NB)]
                    for t in range(T):
                        mk = mkp.tile([128, n], bf16, tag="mk")
                        nc.gpsimd.dma_start(out=mk[:], in_=mask3[t])
                        nc.vector.reduce_sum(out=cntc[:, t:t + 1], in_=mk[:],
                                             axis=mybir.AxisListType.X)
                        for nb in range(NB):
                            nc.tensor.matmul(
                                Wb[nb][:], mh_hi[:, t, :], mk[:, ts(nb, 512)],
                                start=(t == 0), stop=(t == T - 1))

                    # ---- phase 2a: Q = base .* W (frees PSUM banks) ----
                    qs = []
                    for nb in range(NB):
                        q = qp.tile([128, 512], bf16, tag=f"q{nb}")
                        nc.vector.tensor_mul(q[:], Wb[nb][:], sb_base[:, ts(nb, 512)])
                        qs.append(q)

                # ---- phase 2b: V[n] = sum_c Q, masked_sum, count ----
                v_row = rows.tile([1, n], f32, tag="v_row")
                with tc.tile_pool(name=f"psV_{rep}", bufs=2, space="PSUM") as psV:
                    for nb in range(NB):
                        pv = psV.tile([1, 512], f32, tag="pv")
                        nc.tensor.matmul(pv[:], ones_bf[:], qs[nb][:],
                                         start=True, stop=True)
                        nc.vector.tensor_copy(out=v_row[0:1, ts(nb, 512)], in_=pv[:])
                    vs = rows.tile([1, n], f32, tag="vs")
                    nc.vector.tensor_mul(vs[:], v_row[:], rnb_row[:])
                    nc.vector.reduce_sum(out=outpair[0:1, 2 * rep:2 * rep + 1],
                                         in_=vs[:], axis=mybir.AxisListType.X)
                    cnt1 = rows.tile([128, 1], f32, tag="cnt1")
                    nc.vector.reduce_sum(out=cnt1[:], in_=cntc[:],
                                         axis=mybir.AxisListType.X)
                    pc = psV.tile([1, 1], f32, tag="pc")
                    nc.tensor.matmul(pc[:], onescol[:], cnt1[:], start=True, stop=True)
                    nc.vector.tensor_copy(out=outpair[0:1, 2 * rep + 1:2 * rep + 2],
                                          in_=pc[:])

            nc.sync.dma_start(out=out_d[:], in_=outpair[:])

    nc.compile()
    return nc


class SpmdRunner:
    """Compile-once PJRT runner; keeps staged inputs on device."""

    def __init__(self, nc, n_cores):
        import jax
        from jax.sharding import Mesh, PartitionSpec
        from jax.experimental.shard_map import shard_map
        import concourse.mybir as mybir
        from concourse.bass2jax import (_bass_exec_p, install_neuronx_cc_hook,
                                        partition_id_tensor)
        install_neuronx_cc_hook()
        self.jax = jax
        self.PartitionSpec = PartitionSpec
        self.n_cores = n_cores
        in_names, out_names, out_avals, zero_outs = [], [], [], []
        partition_name = (nc.partition_id_tensor.name
                          if nc.partition_id_tensor else None)
        for alloc in nc.m.functions[0].allocations:
            if not isinstance(alloc, mybir.MemoryLocationSet):
                continue
            name = alloc.memorylocations[0].name
            if alloc.kind == "ExternalInput":
                if name != partition_name:
                    in_names.append(name)
            elif alloc.kind == "ExternalOutput":
                out_names.append(name)
                shape = tuple(alloc.tensor_shape)
                dtype = mybir.dt.np(alloc.dtype)
                out_avals.append(jax.core.ShapedArray(shape, dtype))
                zero_outs.append(np.zeros(shape, dtype))
        self.in_names, self.out_names = in_names, out_names
        self.zero_outs = zero_outs
        n_params = len(in_names)
        all_in_names = in_names + out_names
        if partition_name is not None:
            all_in_names.append(partition_name)

        def _body(*args):
            operands = list(args)
            if partition_name is not None:
                operands.append(partition_id_tensor())
            outs = _bass_exec_p.bind(
                *operands,
                out_avals=tuple(out_avals),
                in_names=tuple(all_in_names),
                out_names=tuple(out_names),
                lowering_input_output_aliases=(),
                sim_require_finite=True,
                sim_require_nnan=True,
                nc=nc,
            )
            return tuple(outs)

        devices = jax.devices()[:n_cores]
        self.mesh = Mesh(np.asarray(devices), ("core",))
        in_specs = (PartitionSpec("core"),) * (n_params + len(out_names))
        out_specs = (PartitionSpec("core"),) * len(out_names)
        self.fn = jax.jit(shard_map(_body, mesh=self.mesh, in_specs=in_specs,
                                    out_specs=out_specs, check_rep=False))

    def stage(self, in_maps):
        from jax.sharding import NamedSharding
        args = []
        for name in self.in_names:
            glob = np.concatenate([np.asarray(m[name]) for m in in_maps], axis=0)
            args.append(self.jax.device_put(
                glob, NamedSharding(self.mesh, self.PartitionSpec("core"))))
        for z in self.zero_outs:
            glob = np.concatenate([z] * self.n_cores, axis=0)
            args.append(self.jax.device_put(
                glob, NamedSharding(self.mesh, self.PartitionSpec("core"))))
        return args

    def run(self, args):
        outs = self.fn(*args)
        self.jax.block_until_ready(outs)
        return outs

    def results(self, outs):
        res = [dict() for _ in range(self.n_cores)]
        for i, name in enumerate(self.out_names):
            glob = np.asarray(outs[i])
            per = np.split(glob, self.n_cores, axis=0)
            for c in range(self.n_cores):
                res[c][name] = per[c]
        return res


def make_in_maps(en_base, en_momentum, matrix):
    """Slice full inputs per core: core k -> (batch k//2, m-half k%2)."""
    in_maps = []
    for k in range(N_CORES):
        b, h = k // 2, k % 2
        base = np.ascontiguousarray(en_base[b].reshape(C, HW))
        mom = np.ascontiguousarray(
            en_momentum[b].reshape(C, HW)[:, h * M_LOC:(h + 1) * M_LOC])
        mask = matrix[b, h * M_LOC:(h + 1) * M_LOC, :]
        in_maps.append({"base": base, "mom": mom, "mask": mask})
    return in_maps


def _get_runner():
    global _RUNNER
    if _RUNNER is None:
        nc = build_nc()
        _RUNNER = SpmdRunner(nc, N_CORES)
    return _RUNNER


def kernel(en_base, en_momentum, matrix):
    runner = _get_runner()
    args = runner.stage(make_in_maps(en_base, en_momentum, matrix))
    res = runner.results(runner.run(args))
    tot = np.zeros(2, dtype=np.float64)
    for c in range(N_CORES):
        tot += res[c]["out"][0, :2].astype(np.float64)
    loss = -(tot[0] / tot[1])
    return np.array(loss, dtype=np.float32)


# revision 11
# speedup vs baseline: 14.5278x; 13.9154x over previous
"""Trainium2 Bass kernel for nn_ConsistencyLoss.

loss = -mean(masked_select(cos_sim(mom[b,:,m], base[b,:,n]), mask))

Reformulation so the 268MB int32 mask is streamed through the tensor engine
exactly once (memory-bound regime):

    masked_sum = sum_{b,m,n} M[b,m,n] * (mhat_m . bhat_n)
               = sum_{b,c,n} bhat[c,n] * W[c,n],   W = mhatT.T @ M   (contract over m)
    loss       = -masked_sum / sum(M)

Sharding: 8 cores = (batch b in 0..3) x (half of the m rows). Each core:
  - mom slice [C=128, 2048] -> 16 PE transposes into one 4-bank PSUM tile,
    one batched DVE square+reduce for the row norms, then 16 ACT copies that
    fold the 1/||mom_m|| scale into the PSUM->SBUF bf16 cast (mhat in bf16
    costs ~1e-3 rel err on the loss, far under the 2e-2 gate)
  - mask slice [2048, 4096] int32, streamed as 16 x [128,4096] cast-DMAs
    (int32 -> bf16 in the DMA engine, 8-deep buffer pool) feeding rhs of
    accumulating matmuls; W accumulated in 8 PSUM banks [128 c, 512 n]
  - tail: Q = base .* W (bf16), V[n] = sum_c Q (ones-matmul),
    masked_sum = (V .* 1/||base_n||) summed; count from per-tile DVE reduces
    of the bf16 mask (exact in fp32)
Host combines 8 (sum, count) pairs.
"""

import sys

for _p in ("/opt/trn_rl_repo",):
    if _p not in sys.path:
        sys.path.insert(0, _p)

import numpy as np

B, C, HW = 4, 128, 4096          # batch, channels, H*W
M_LOC = HW // 2                  # momentum rows per core
N_CORES = 8

_RUNNER = None


def build_nc(n=HW, m_loc=M_LOC, n_cores=N_CORES, reps=1, variant=""):
    """Build the per-core Bass module. n = base pixels, m_loc = local momentum rows.

    variant is for benchmarking ablations only ("" = the real kernel):
    "notail" skips phases 2a/2b, "nophase0" stubs the normalization prep.
    """
    do_phase0 = variant != "nophase0"
    do_tail = variant != "notail"
    import concourse.bass as bass
    import concourse.mybir as mybir
    import concourse.tile as tile
    from concourse import bacc
    from concourse.masks import make_identity
    from concourse.bass import ts

    f32 = mybir.dt.float32
    bf16 = mybir.dt.bfloat16
    i32 = mybir.dt.int32
    T = m_loc // 128             # mask row tiles
    NB = n // 512                # 512-wide n blocks (one PSUM bank each)
    assert NB <= 8

    nc = bacc.Bacc("TRN2", target_bir_lowering=False, debug=False,
                   num_devices=n_cores)
    base_d = nc.declare_dram_parameter("base", [128, n], f32, isOutput=False)
    mom_d = nc.declare_dram_parameter("mom", [128, m_loc], f32, isOutput=False)
    mask_d = nc.declare_dram_parameter("mask", [m_loc, n], i32, isOutput=False)
    out_d = nc.declare_dram_parameter("out", [1, 2 * reps], f32, isOutput=True)
    mask3 = mask_d.rearrange("(t p) n -> t p n", p=128)

    with tile.TileContext(nc) as tc:
        from contextlib import ExitStack
        with ExitStack() as ctx:
            ctx.enter_context(nc.allow_low_precision(
                reason="bf16 intermediates validated vs fp32 reference (~1e-3)"))
            sing = ctx.enter_context(tc.tile_pool(name="sing", bufs=1))
            sb = ctx.enter_context(tc.tile_pool(name="sb", bufs=2))
            sb1 = ctx.enter_context(tc.tile_pool(name="sb1", bufs=2))
            rows = ctx.enter_context(tc.tile_pool(name="rows", bufs=1))
            mkbp = ctx.enter_context(tc.tile_pool(name="mkb", bufs=3))
            mip = ctx.enter_context(tc.tile_pool(name="mi", bufs=3))
            mkcp = ctx.enter_context(tc.tile_pool(name="mkc", bufs=3))
            qp = ctx.enter_context(tc.tile_pool(name="qp", bufs=1))

            ident = sing.tile([128, 128], f32)
            make_identity(nc, ident)
            onescol = sing.tile([128, 1], f32)
            nc.vector.memset(onescol, 1.0)
            ones_bf = sing.tile([128, 1], bf16)
            nc.vector.memset(ones_bf, 1.0)

            outpair = sing.tile([1, 2 * reps], f32)

            for rep in range(reps):
                sb_base = sb.tile([128, n], f32, tag="sb_base")
                nc.scalar.dma_start(out=sb_base[:], in_=base_d[:])
                sb_mom = rows.tile([128, m_loc], f32, tag="sb_mom")
                nc.scalar.dma_start(out=sb_mom[:], in_=mom_d[:])

                mh_hi = sb1.tile([128, T, 128], bf16, tag="mh_hi")
                cntc = sb1.tile([128, T], f32, tag="cntc")

                # ---- phase 0a: batched mom transpose + row norms ----
                if do_phase0:
                    with tc.tile_pool(name=f"ps0_{rep}", bufs=1, space="PSUM") as ps0:
                        tp_all = ps0.tile([128, T, 128], f32, tag="tp_all")
                        for t in range(T):
                            nc.tensor.transpose(tp_all[:, t, :], sb_mom[:, ts(t, 128)],
                                                ident[:])
                        sqd = qp.tile([128, 128], bf16, tag="sqd")
                        nsq = sb1.tile([128, T], f32, tag="nsq")
                        for t in range(T):
                            nc.scalar.activation(out=sqd[:], in_=tp_all[:, t, :],
                                                 func=mybir.ActivationFunctionType.Square,
                                                 accum_out=nsq[:, t:t + 1])
                        rn_a = sb1.tile([128, T], f32, tag="rn_a")
                        nc.vector.reciprocal(out=rn_a[:], in_=nsq[:])
                        rn = sb1.tile([128, T], f32, tag="rn")
                        nc.scalar.activation(out=rn[:], in_=rn_a[:],
                                             func=mybir.ActivationFunctionType.Sqrt)
                        for t in range(T):
                            nc.scalar.mul(mh_hi[:, t, :], tp_all[:, t, :],
                                          rn[:, t:t + 1])

                    # ---- phase 0b: norm_b (column norms of base, bf16 squares) ----
                    sqb = qp.tile([128, n], bf16, tag="sqb")
                    nc.scalar.activation(out=sqb[:], in_=sb_base[:],
                                         func=mybir.ActivationFunctionType.Square)
                    nb2_row = rows.tile([1, n], bf16, tag="nb2")
                    rnb_a = rows.tile([1, n], bf16, tag="rnb_a")
                    rnb_row = rows.tile([1, n], bf16, tag="rnb")
                    with tc.tile_pool(name=f"psnb_{rep}", bufs=2, space="PSUM") as psnb:
                        for nb in range(NB):
                            pnb = psnb.tile([1, 512], f32, tag="pnb")
                            nc.tensor.matmul(pnb[:], ones_bf[:], sqb[:, ts(nb, 512)],
                                             start=True, stop=True)
                            nc.vector.tensor_copy(out=nb2_row[0:1, ts(nb, 512)],
                                                  in_=pnb[:])
                    nc.vector.reciprocal(out=rnb_a[:], in_=nb2_row[:])
                    nc.scalar.activation(out=rnb_row[:], in_=rnb_a[:],
                                         func=mybir.ActivationFunctionType.Sqrt)
                else:
                    nc.vector.memset(mh_hi, 0.001)
                    rnb_row = rows.tile([1, n], bf16, tag="rnb")
                    nc.vector.memset(rnb_row, 1.0)

                # ---- phase 1: stream mask, accumulate W in PSUM ----
                with tc.tile_pool(name=f"psW_{rep}", bufs=1, space="PSUM") as psW:
                    Wb = [psW.tile([128, 512], f32, tag=f"w{nb}", name=f"w{nb}")
                          for nb in range(NB)]
                    for t in range(T):
                        if t % 2 == 0:
                            # SWDGE path: int32 -> bf16 cast in the DMA engine
                            mk = mkbp.tile([128, n], bf16, tag="mk")
                            nc.gpsimd.dma_start(out=mk[:], in_=mask3[t])
                            nc.vector.reduce_sum(out=cntc[:, t:t + 1], in_=mk[:],
                                                 axis=mybir.AxisListType.X)
                        else:
                            # HWDGE path: raw int32, cast on ACT (fused count)
                            # or DVE (separate count)
                            mi = mip.tile([128, n], i32, tag="mi")
                            nc.sync.dma_start(out=mi[:], in_=mask3[t])
                            mk = mkcp.tile([128, n], bf16, tag="mkc")
                            if t % 4 == 1:
                                nc.scalar.activation(
                                    out=mk[:], in_=mi[:],
                                    func=mybir.ActivationFunctionType.Copy,
                                    accum_out=cntc[:, t:t + 1])
                            else:
                                nc.vector.tensor_copy(out=mk[:], in_=mi[:])
                                nc.vector.reduce_sum(out=cntc[:, t:t + 1],
                                                     in_=mk[:],
                                                     axis=mybir.AxisListType.X)
                        for nb in range(NB):
                            nc.tensor.matmul(
                                Wb[nb][:], mh_hi[:, t, :], mk[:, ts(nb, 512)],
                                start=(t == 0), stop=(t == T - 1))

                    # ---- phase 2a: Q = base .* W (frees PSUM banks) ----
                    qs = []
                    if do_tail:
                        for nb in range(NB):
                            q = qp.tile([128, 512], bf16, tag=f"q{nb}")
                            nc.vector.tensor_mul(q[:], Wb[nb][:],
                                                 sb_base[:, ts(nb, 512)])
                            qs.append(q)

                # ---- phase 2b: V[n] = sum_c Q, masked_sum, count ----
                with tc.tile_pool(name=f"psV_{rep}", bufs=2, space="PSUM") as psV:
                    if do_tail:
                        v_row = rows.tile([1, n], bf16, tag="v_row")
                        for nb in range(NB):
                            pv = psV.tile([1, 512], f32, tag="pv")
                            nc.tensor.matmul(pv[:], ones_bf[:], qs[nb][:],
                                             start=True, stop=True)
                            nc.vector.tensor_copy(out=v_row[0:1, ts(nb, 512)],
                                                  in_=pv[:])
                        vs = rows.tile([1, n], bf16, tag="vs")
                        nc.vector.tensor_mul(vs[:], v_row[:], rnb_row[:])
                        nc.vector.reduce_sum(out=outpair[0:1, 2 * rep:2 * rep + 1],
                                             in_=vs[:], axis=mybir.AxisListType.X)
                    cnt1 = rows.tile([128, 1], f32, tag="cnt1")
                    nc.vector.reduce_sum(out=cnt1[:], in_=cntc[:],
                                         axis=mybir.AxisListType.X)
                    pc = psV.tile([1, 1], f32, tag="pc")
                    nc.tensor.matmul(pc[:], onescol[:], cnt1[:], start=True, stop=True)
                    if not do_tail:
                        nc.vector.tensor_copy(out=outpair[0:1, 2 * rep:2 * rep + 1],
                                              in_=pc[:])
                    nc.vector.tensor_copy(out=outpair[0:1, 2 * rep + 1:2 * rep + 2],
                                          in_=pc[:])

            nc.sync.dma_start(out=out_d[:], in_=outpair[:])

    nc.compile()
    return nc


class SpmdRunner:
    """Compile-once PJRT runner; keeps staged inputs on device."""

    def __init__(self, nc, n_cores):
        import jax
        from jax.sharding import Mesh, PartitionSpec
        from jax.experimental.shard_map import shard_map
        import concourse.mybir as mybir
        from concourse.bass2jax import (_bass_exec_p, install_neuronx_cc_hook,
                                        partition_id_tensor)
        install_neuronx_cc_hook()
        self.jax = jax
        self.PartitionSpec = PartitionSpec
        self.n_cores = n_cores
        in_names, out_names, out_avals, zero_outs = [], [], [], []
        partition_name = (nc.partition_id_tensor.name
                          if nc.partition_id_tensor else None)
        for alloc in nc.m.functions[0].allocations:
            if not isinstance(alloc, mybir.MemoryLocationSet):
                continue
            name = alloc.memorylocations[0].name
            if alloc.kind == "ExternalInput":
                if name != partition_name:
                    in_names.append(name)
            elif alloc.kind == "ExternalOutput":
                out_names.append(name)
                shape = tuple(alloc.tensor_shape)
                dtype = mybir.dt.np(alloc.dtype)
                out_avals.append(jax.core.ShapedArray(shape, dtype))
                zero_outs.append(np.zeros(shape, dtype))
        self.in_names, self.out_names = in_names, out_names
        self.zero_outs = zero_outs
        n_params = len(in_names)
        all_in_names = in_names + out_names
        if partition_name is not None:
            all_in_names.append(partition_name)

        def _body(*args):
            operands = list(args)
            if partition_name is not None:
                operands.append(partition_id_tensor())
            outs = _bass_exec_p.bind(
                *operands,
                out_avals=tuple(out_avals),
                in_names=tuple(all_in_names),
                out_names=tuple(out_names),
                lowering_input_output_aliases=(),
                sim_require_finite=True,
                sim_require_nnan=True,
                nc=nc,
            )
            return tuple(outs)

        devices = jax.devices()[:n_cores]
        self.mesh = Mesh(np.asarray(devices), ("core",))
        in_specs = (PartitionSpec("core"),) * (n_params + len(out_names))
        out_specs = (PartitionSpec("core"),) * len(out_names)
        self.fn = jax.jit(shard_map(_body, mesh=self.mesh, in_specs=in_specs,
                                    out_specs=out_specs, check_rep=False))

    def stage(self, in_maps):
        from jax.sharding import NamedSharding
        args = []
        for name in self.in_names:
            glob = np.concatenate([np.asarray(m[name]) for m in in_maps], axis=0)
            args.append(self.jax.device_put(
                glob, NamedSharding(self.mesh, self.PartitionSpec("core"))))
        for z in self.zero_outs:
            glob = np.concatenate([z] * self.n_cores, axis=0)
            args.append(self.jax.device_put(
                glob, NamedSharding(self.mesh, self.PartitionSpec("core"))))
        return args

    def run(self, args):
        outs = self.fn(*args)
        self.jax.block_until_ready(outs)
        return outs

    def results(self, outs):
        res = [dict() for _ in range(self.n_cores)]
        for i, name in enumerate(self.out_names):
            glob = np.asarray(outs[i])
            per = np.split(glob, self.n_cores, axis=0)
            for c in range(self.n_cores):
                res[c][name] = per[c]
        return res


def make_in_maps(en_base, en_momentum, matrix):
    """Slice full inputs per core: core k -> (batch k//2, m-half k%2)."""
    in_maps = []
    for k in range(N_CORES):
        b, h = k // 2, k % 2
        base = np.ascontiguousarray(en_base[b].reshape(C, HW))
        mom = np.ascontiguousarray(
            en_momentum[b].reshape(C, HW)[:, h * M_LOC:(h + 1) * M_LOC])
        mask = matrix[b, h * M_LOC:(h + 1) * M_LOC, :]
        in_maps.append({"base": base, "mom": mom, "mask": mask})
    return in_maps


def _get_runner():
    global _RUNNER
    if _RUNNER is None:
        nc = build_nc()
        _RUNNER = SpmdRunner(nc, N_CORES)
    return _RUNNER


def kernel(en_base, en_momentum, matrix):
    runner = _get_runner()
    args = runner.stage(make_in_maps(en_base, en_momentum, matrix))
    res = runner.results(runner.run(args))
    tot = np.zeros(2, dtype=np.float64)
    for c in range(N_CORES):
        tot += res[c]["out"][0, :2].astype(np.float64)
    loss = -(tot[0] / tot[1])
    return np.array(loss, dtype=np.float32)
